# revision 11
# baseline (speedup 1.0000x reference)
"""Trainium2 Bass kernel for nn_BucketedGoWatti (sparse windowed attention pooling).

Math (B=4, L=4096, T=32, DH=1024, DG=256, DP=256, WIN=1024, STRIDE=256, W=13):
  All 13 windows are unions of 4 consecutive 256-wide l-chunks, and the
  per-window logits are slices of one global score matrix
    S[b,t,l] = (qk_b @ H_b^T)[t,l],  qk = (G Wq_core) Wk_core^T * DP^-0.5.
  So the device only computes, per 256-chunk c:
    E = exp(S),  s_c[t] = sum_{l in c} E[t,l],  P_c[t,:] = E[t, c] @ H[c, :]
  and the tiny 13-window combine (Zw = sum4(P)/sum4(s), cross-window softmax
  with qw2 = (G Wq_win) Wk_win^T * DH^-0.5) runs on host in f32.

Sharding: core = 2*b + half owns l in [half*2048, half*2048+2048) of batch b
(8 disjoint 256-chunks). H ships in fp16; the transposed layout needed by S
is only partially shipped (HBM is the bottleneck): chunk 0's H^T is built
fully on-chip by PE transposes from the natural tiles, chunks 1-2 half, and
chunk 3 ships whole so the last-arrival tail stays short.

PE shape: T=32 output rows -> 4x column tiling, so S and P run as 4
concurrent matmuls; the cross-col-group S reduce is one f32r matmul against
a [128,32] stacked identity. Numpy-sim rel err ~4e-4 (gate 2e-2).
"""
import numpy as np
from contextlib import ExitStack

import concourse.bacc as bacc
import concourse.tile as tile
import concourse.mybir as mybir
import concourse.masks as masks
from concourse.bass_utils import run_bass_kernel_spmd

F32 = mybir.dt.float32
F32R = mybir.dt.float32r
F16 = mybir.dt.float16
ActFn = mybir.ActivationFunctionType

B, L, T = 4, 4096, 32
DH, DG, DP = 1024, 256, 256
WIN, STRIDE = 1024, 256
W = (L - WIN) // STRIDE + 1          # 13
SPAN = 2048                          # per-core l-span
NLC = SPAN // 512                    # 4 l-chunks of 512
NCH = SPAN // 256                    # 8 chunks of 256 (the window quanta)
S_CORE = 1.0 / float(np.sqrt(DP))
S_WIN = 1.0 / float(np.sqrt(DH))

_CACHE = {}


def _build(with_mask: bool):
    nc = bacc.Bacc("TRN2", debug=False, target_bir_lowering=False)

    # natural layout [j, p, s*1024+d] with l = j*512 + s*128 + p
    Hn_d = nc.dram_tensor("Hn", [NLC, 128, 4096], F16, kind="ExternalInput")
    # shipped transposed parts: lc 1,2 d-tiles 0-3; lc 3 all 8
    HTq_d = nc.dram_tensor("HTq", [1, 128, 2048], F16, kind="ExternalInput")
    HT8_d = nc.dram_tensor("HT8", [128, 4096], F16, kind="ExternalInput")
    # [p, i*32+t] = qkT[i*128+p, t]
    QKT_d = nc.dram_tensor("QKT", [128, 256], F16, kind="ExternalInput")
    SEL_d = nc.dram_tensor("SEL", [128, 32], F32R, kind="ExternalInput")
    if with_mask:
        mb_d = nc.dram_tensor("maskbias", [1, SPAN], F32R, kind="ExternalInput")
        ones_d = nc.dram_tensor("onesrow", [1, T], F32R, kind="ExternalInput")
    # row 32*(2*cc+h)+t, col lc*512+d'  ->  P[2lc+cc][t, h*512+d']
    P_d = nc.dram_tensor("P_out", [128, NLC * 512], F16, kind="ExternalOutput")
    S_d = nc.dram_tensor("ssum_out", [T, NCH], F32, kind="ExternalOutput")

    with tile.TileContext(nc) as tc, ExitStack() as ctx:
        const = ctx.enter_context(tc.tile_pool(name="const", bufs=1))
        hpool = ctx.enter_context(tc.tile_pool(name="hpool", bufs=4))
        htqp = ctx.enter_context(tc.tile_pool(name="htqp", bufs=2))
        hthf = ctx.enter_context(tc.tile_pool(name="hthf", bufs=2))
        hthp = ctx.enter_context(tc.tile_pool(name="hthp", bufs=1))
        ep = ctx.enter_context(tc.tile_pool(name="ep", bufs=2))
        spl = ctx.enter_context(tc.tile_pool(name="spl", bufs=2))
        etp = ctx.enter_context(tc.tile_pool(name="etp", bufs=2))
        sb = ctx.enter_context(tc.tile_pool(name="sb", bufs=1))
        sp4 = ctx.enter_context(tc.tile_pool(name="sp4", bufs=2, space="PSUM"))
        lg = ctx.enter_context(tc.tile_pool(name="lg", bufs=2, space="PSUM"))
        pj = ctx.enter_context(tc.tile_pool(name="pj", bufs=2, space="PSUM"))
        zp = ctx.enter_context(tc.tile_pool(name="zp", bufs=2, space="PSUM"))

        ident = const.tile([128, 128], F32, tag="ident")
        masks.make_identity(nc, ident[:])
        identh = const.tile([128, 128], F16, tag="identh")
        nc.vector.tensor_copy(identh[:], ident[:])

        sel = const.tile([128, 32], F32R, tag="sel")
        nc.scalar.dma_start(sel[:], SEL_d.ap())
        qkt = const.tile([128, 256], F16, tag="qkt")
        nc.scalar.dma_start(qkt[:], QKT_d.ap())
        if with_mask:
            mbias = const.tile([1, SPAN], F32R, tag="mbias")
            onesr = const.tile([1, T], F32R, tag="onesr")
            nc.gpsimd.dma_start(mbias[:], mb_d.ap())
            nc.gpsimd.dma_start(onesr[:], ones_d.ap())

        # big input stream, ordered by first need
        hn = []
        for lc in range(NLC):
            t2_ = hpool.tile([128, 4096], F16, tag="hn")
            hn.append(t2_)
        htq = {}
        nc.sync.dma_start(hn[0][:], Hn_d.ap()[0])
        nc.sync.dma_start(hn[1][:], Hn_d.ap()[1])
        ht8 = const.tile([128, 4096], F16, tag="ht8")
        nc.sync.dma_start(ht8[:], HT8_d.ap())
        nc.sync.dma_start(hn[2][:], Hn_d.ap()[2])
        t_ = htqp.tile([128, 2048], F16, tag="htq")
        nc.sync.dma_start(t_[:], HTq_d.ap()[0])
        htq[2] = t_
        nc.sync.dma_start(hn[3][:], Hn_d.ap()[3])

        # ~4us of dummy f32 matmuls while the first DMAs stream in: trips the
        # PE HAM un-throttle (~3.4us sustained busy) so later matmuls run at
        # 2.4GHz instead of the cold 1.2GHz default.
        warm = sp4.tile([128, 512], F32, tag="spart")
        for _ in range(12):
            nc.tensor.matmul(warm[:, :128], ident[:], ident[:],
                             start=True, stop=True)

        hth = {}

        def h_tr(lc, i0, n, dst):
            # build HT d-tiles [i0, i0+n) on-chip from natural tiles
            for ii in range(i0, i0 + n, 2):
                pst = pj.tile([128, 1024], F16, tag="pj")
                for m in range(2):
                    i = ii + m
                    for s in range(4):
                        nc.tensor.transpose(
                            pst[:, (m * 4 + s) * 128:(m * 4 + s + 1) * 128],
                            hn[lc][:, s * 1024 + i * 128:s * 1024 + (i + 1) * 128],
                            identh[:])
                nc.vector.tensor_copy(
                    dst[:, (ii - i0) * 512:(ii - i0 + 2) * 512], pst[:])

        def s_rhs(lc, i):
            if lc in (0, 1):
                return hth[lc][:, i * 512:(i + 1) * 512]
            if lc == 3:
                return ht8[:, i * 512:(i + 1) * 512]
            if i < 4:
                return htq[lc][:, i * 512:(i + 1) * 512]
            return hth[lc][:, (i - 4) * 512:(i - 3) * 512]

        pout = sb.tile([128, NLC * 512], F16, tag="pout")
        ssum = sb.tile([T, NCH], F32, tag="ssum")
        es = {}

        def s_phase(lc):
            if lc in (0, 1):
                hth[lc] = hthf.tile([128, 4096], F16, tag="hthf", name="hthf")
                h_tr(lc, 0, 8, hth[lc])
            elif lc == 2:
                hth[lc] = hthp.tile([128, 2048], F16, tag="hth", name="hth")
                h_tr(lc, 4, 4, hth[lc])
            # --- S[t, l]: 4 col-groups x 2 d-tiles each ---
            spart = sp4.tile([128, 512], F32, tag="spart")
            for k in range(2):
                for g in range(4):
                    i = g + 4 * k
                    nc.tensor.matmul(spart[32 * g:32 * g + 32, :],
                                     qkt[:, i * 32:(i + 1) * 32], s_rhs(lc, i),
                                     start=(k == 0), stop=(k == 1),
                                     tile_position=(0, 32 * g))
            sparts = spl.tile([128, 512], F32R, tag="sps")
            nc.vector.tensor_copy(sparts[:], spart[:])
            sps = lg.tile([T, 512], F32, tag="lg")
            nc.tensor.matmul(sps[:], sel[:], sparts[:],
                             start=True, stop=(not with_mask))
            if with_mask:
                nc.tensor.matmul(sps[:], onesr[:],
                                 mbias[:, lc * 512:(lc + 1) * 512],
                                 start=False, stop=True)
            # --- E = exp(S) in fp16, with per-256-chunk row sums ---
            e_ = ep.tile([T, 512], F16, tag="e")
            nc.scalar.activation(e_[:], sps[:], ActFn.Exp)
            for u in range(2):
                c = 2 * lc + u
                nc.vector.reduce_sum(ssum[:, c:c + 1],
                                     e_[:, u * 256:(u + 1) * 256],
                                     axis=mybir.AxisListType.X)
            es[lc] = e_

        ets = {}

        def t_phase(lc):
            # --- E^T fp16: 4 PE transposes packed in one PSUM tile, 1 copy ---
            e_ = es[lc]
            pst = pj.tile([128, 1024], F16, tag="pj")
            for k in range(4):
                nc.tensor.transpose(pst[:, k * 32:(k + 1) * 32],
                                    e_[:, k * 128:(k + 1) * 128],
                                    identh[:32, :32])
            et = etp.tile([128, 128], F16, tag="et")
            nc.vector.tensor_copy(et[:], pst[:, :128])
            ets[lc] = et

        def p_phase(lc):
            # --- P: 4 col-groups j=(cc,h), 2 l-subtiles each ---
            et = ets[lc]
            ppack = zp.tile([128, 512], F32, tag="zp")
            for k in range(2):
                for j in range(4):
                    cc, h = j // 2, j % 2
                    s = 2 * cc + k
                    nc.tensor.matmul(ppack[32 * j:32 * j + 32, :],
                                     et[:, s * 32:(s + 1) * 32],
                                     hn[lc][:, s * 1024 + h * 512:
                                            s * 1024 + h * 512 + 512],
                                     start=(k == 0), stop=(k == 1),
                                     tile_position=(0, 32 * j))
            if lc < 3:
                nc.scalar.activation(pout[:, lc * 512:(lc + 1) * 512], ppack[:],
                                     ActFn.Identity)
            else:
                nc.vector.tensor_copy(pout[:, lc * 512:(lc + 1) * 512], ppack[:])
            # ship this l-chunk's quarter of P as soon as its copy lands
            nc.scalar.dma_start(P_d.ap()[:, lc * 512:(lc + 1) * 512],
                                pout[:, lc * 512:(lc + 1) * 512])

        s_phase(0)
        s_phase(1)
        t_phase(0)
        p_phase(0)
        s_phase(2)
        t_phase(1)
        p_phase(1)
        s_phase(3)
        t_phase(2)
        p_phase(2)
        t_phase(3)
        nc.scalar.dma_start(S_d.ap(), ssum[:])
        p_phase(3)

    nc.compile()
    return nc


def kernel(H, G, Wq_core, Wk_core, Wq_win, Wk_win, attn_mask):
    H = np.asarray(H, dtype=np.float32)
    G = np.asarray(G, dtype=np.float32)
    Wq_core = np.asarray(Wq_core, dtype=np.float32)
    Wk_core = np.asarray(Wk_core, dtype=np.float32)
    Wq_win = np.asarray(Wq_win, dtype=np.float32)
    Wk_win = np.asarray(Wk_win, dtype=np.float32)
    mask = np.asarray(attn_mask).astype(bool)

    with_mask = not bool(mask.all())
    key = ("k", with_mask)
    if key not in _CACHE:
        _CACHE[key] = _build(with_mask)
    nc = _CACHE[key]

    # host-side tiny G projections (weight-space only, no H involvement)
    qk = (G @ Wq_core) @ Wk_core.T * S_CORE          # [B, T, DH]
    qw2 = (G @ Wq_win) @ Wk_win.T * S_WIN            # [B, T, DH]
    selmat = np.tile(np.eye(32, dtype=np.float32), (4, 1))  # [128, 32]

    in_maps = []
    for c in range(8):
        b, half = c // 2, c % 2
        l0 = half * SPAN
        H16 = H[b, l0:l0 + SPAN, :].astype(np.float16)          # [2048, 1024]
        HT16 = np.ascontiguousarray(H[b].T[:, l0:l0 + SPAN]).astype(np.float16)
        # [i, p, lc, f] -> [lc, p, i, f]
        HTr = np.ascontiguousarray(
            HT16.reshape(8, 128, NLC, 512).transpose(2, 1, 0, 3)
        ).reshape(NLC, 128, 4096)
        # [j, s, p, f] -> [j, p, s, f]
        Hnr = np.ascontiguousarray(
            H16.reshape(NLC, 4, 128, DH).transpose(0, 2, 1, 3)
        ).reshape(NLC, 128, 4096)
        qkT16 = qk[b].T.astype(np.float16)                      # [1024, 32]
        QKTr = np.ascontiguousarray(
            qkT16.reshape(8, 128, 32).transpose(1, 0, 2)
        ).reshape(128, 256)
        im = {"Hn": Hnr, "HTq": np.ascontiguousarray(HTr[2:3, :, :2048]),
              "HT8": HTr[3], "QKT": QKTr, "SEL": selmat}
        if with_mask:
            im["maskbias"] = np.where(mask[b, l0:l0 + SPAN], 0.0,
                                      -1e9).astype(np.float32)[None, :]
            im["onesrow"] = np.ones((1, T), dtype=np.float32)
        in_maps.append(im)

    import os
    prof_dir = os.environ.get("BGW_PROFILE_DIR")
    res = None
    if prof_dir:
        try:
            res = run_bass_kernel_spmd(nc, in_maps, core_ids=list(range(8)),
                                       trace=True, tmpdir=prof_dir)
        except Exception:
            res = None
    if res is None:
        res = run_bass_kernel_spmd(nc, in_maps, core_ids=list(range(8)))
    kernel._last_result = res

    # ---- host combine: windows = sums of 4 chunk partials, tiny softmax ----
    NCHB = L // 256                                   # 16 chunks per batch
    Z = np.empty((B, T, DH), dtype=np.float32)
    for b in range(B):
        P = np.empty((NCHB, T, DH), dtype=np.float32)
        ss = np.empty((NCHB, T), dtype=np.float32)
        for half in range(2):
            r = res.results[2 * b + half]
            arr = r["P_out"].astype(np.float32).reshape(4, 32, NLC, 512)
            for lc in range(NLC):
                for cc in range(2):
                    for h in range(2):
                        P[half * NCH + 2 * lc + cc, :, h * 512:(h + 1) * 512] = \
                            arr[2 * cc + h, :, lc, :]
            ss[half * NCH:(half + 1) * NCH] = r["ssum_out"].T
        Zw = np.empty((W, T, DH), dtype=np.float32)
        wlog = np.empty((T, W), dtype=np.float32)
        for w in range(W):
            num = P[w] + P[w + 1] + P[w + 2] + P[w + 3]
            den = ss[w] + ss[w + 1] + ss[w + 2] + ss[w + 3]
            Zw[w] = num / den[:, None]
            wlog[:, w] = (Zw[w] * qw2[b]).sum(-1)
        m2 = wlog.max(-1, keepdims=True)
        wsm = np.exp(wlog - m2)
        wsm /= wsm.sum(-1, keepdims=True)
        Z[b] = np.einsum("tw,wtd->td", wsm, Zw)
    return Z


# revision 12
# speedup vs baseline: 1.0291x; 1.0291x over previous
"""Trainium2 Bass kernel for nn_BucketedGoWatti (sparse windowed attention pooling).

Math (B=4, L=4096, T=32, DH=1024, DG=256, DP=256, WIN=1024, STRIDE=256, W=13):
  All 13 windows are unions of 4 consecutive 256-wide l-chunks, and the
  per-window logits are slices of one global score matrix
    S[b,t,l] = (qk_b @ H_b^T)[t,l],  qk = (G Wq_core) Wk_core^T * DP^-0.5.
  So the device only computes, per 256-chunk c:
    E = exp(S),  s_c[t] = sum_{l in c} E[t,l],  P_c[t,:] = E[t, c] @ H[c, :]
  and the tiny 13-window combine (Zw = sum4(P)/sum4(s), cross-window softmax
  with qw2 = (G Wq_win) Wk_win^T * DH^-0.5) runs on host in f32.

Sharding: core = 2*b + half owns l in [half*2048, half*2048+2048) of batch b
(8 disjoint 256-chunks). H ships in fp16; the transposed layout needed by S
is only partially shipped (HBM is the bottleneck): chunk 0's H^T is built
fully on-chip by PE transposes from the natural tiles, chunks 1-2 half, and
chunk 3 ships whole so the last-arrival tail stays short.

PE shape: T=32 output rows -> 4x column tiling, so S and P run as 4
concurrent matmuls; the cross-col-group S reduce is one f32r matmul against
a [128,32] stacked identity. Numpy-sim rel err ~4e-4 (gate 2e-2).
"""
import numpy as np
from contextlib import ExitStack

import concourse.bacc as bacc
import concourse.tile as tile
import concourse.mybir as mybir
import concourse.masks as masks
from concourse.bass_utils import run_bass_kernel_spmd

F32 = mybir.dt.float32
F32R = mybir.dt.float32r
F16 = mybir.dt.float16
ActFn = mybir.ActivationFunctionType

B, L, T = 4, 4096, 32
DH, DG, DP = 1024, 256, 256
WIN, STRIDE = 1024, 256
W = (L - WIN) // STRIDE + 1          # 13
SPAN = 2048                          # per-core l-span
NLC = SPAN // 512                    # 4 l-chunks of 512
NCH = SPAN // 256                    # 8 chunks of 256 (the window quanta)
S_CORE = 1.0 / float(np.sqrt(DP))
S_WIN = 1.0 / float(np.sqrt(DH))

_CACHE = {}


def _build(with_mask: bool):
    nc = bacc.Bacc("TRN2", debug=False, target_bir_lowering=False)

    # natural layout [j, p, s*1024+d] with l = j*512 + s*128 + p
    Hn_d = nc.dram_tensor("Hn", [NLC, 128, 4096], F16, kind="ExternalInput")
    # shipped transposed parts: lc 1,2 d-tiles 0-3; lc 3 all 8
    HTq_d = nc.dram_tensor("HTq", [2, 128, 2048], F16, kind="ExternalInput")
    HT8_d = nc.dram_tensor("HT8", [128, 4096], F16, kind="ExternalInput")
    # [p, i*32+t] = qkT[i*128+p, t]
    QKT_d = nc.dram_tensor("QKT", [128, 256], F16, kind="ExternalInput")
    SEL_d = nc.dram_tensor("SEL", [128, 32], F32R, kind="ExternalInput")
    if with_mask:
        mb_d = nc.dram_tensor("maskbias", [1, SPAN], F32R, kind="ExternalInput")
        ones_d = nc.dram_tensor("onesrow", [1, T], F32R, kind="ExternalInput")
    # row 32*(2*cc+h)+t, col lc*512+d'  ->  P[2lc+cc][t, h*512+d']
    P_d = nc.dram_tensor("P_out", [128, NLC * 512], F16, kind="ExternalOutput")
    S_d = nc.dram_tensor("ssum_out", [T, NCH], F32, kind="ExternalOutput")

    with tile.TileContext(nc) as tc, ExitStack() as ctx:
        const = ctx.enter_context(tc.tile_pool(name="const", bufs=1))
        hpool = ctx.enter_context(tc.tile_pool(name="hpool", bufs=4))
        htqp = ctx.enter_context(tc.tile_pool(name="htqp", bufs=2))
        hthf = ctx.enter_context(tc.tile_pool(name="hthf", bufs=1))
        hthp = ctx.enter_context(tc.tile_pool(name="hthp", bufs=2))
        ep = ctx.enter_context(tc.tile_pool(name="ep", bufs=2))
        spl = ctx.enter_context(tc.tile_pool(name="spl", bufs=2))
        etp = ctx.enter_context(tc.tile_pool(name="etp", bufs=2))
        sb = ctx.enter_context(tc.tile_pool(name="sb", bufs=1))
        sp4 = ctx.enter_context(tc.tile_pool(name="sp4", bufs=2, space="PSUM"))
        lg = ctx.enter_context(tc.tile_pool(name="lg", bufs=2, space="PSUM"))
        pj = ctx.enter_context(tc.tile_pool(name="pj", bufs=2, space="PSUM"))
        zp = ctx.enter_context(tc.tile_pool(name="zp", bufs=2, space="PSUM"))

        ident = const.tile([128, 128], F32, tag="ident")
        masks.make_identity(nc, ident[:])
        identh = const.tile([128, 128], F16, tag="identh")
        nc.vector.tensor_copy(identh[:], ident[:])

        sel = const.tile([128, 32], F32R, tag="sel")
        nc.scalar.dma_start(sel[:], SEL_d.ap())
        qkt = const.tile([128, 256], F16, tag="qkt")
        nc.scalar.dma_start(qkt[:], QKT_d.ap())
        if with_mask:
            mbias = const.tile([1, SPAN], F32R, tag="mbias")
            onesr = const.tile([1, T], F32R, tag="onesr")
            nc.gpsimd.dma_start(mbias[:], mb_d.ap())
            nc.gpsimd.dma_start(onesr[:], ones_d.ap())

        # big input stream, ordered by first need
        hn = []
        for lc in range(NLC):
            t2_ = hpool.tile([128, 4096], F16, tag="hn")
            hn.append(t2_)
        htq = {}
        nc.sync.dma_start(hn[0][:], Hn_d.ap()[0])
        nc.sync.dma_start(hn[1][:], Hn_d.ap()[1])
        for lc in (1, 2):
            t_ = htqp.tile([128, 2048], F16, tag="htq")
            nc.sync.dma_start(t_[:], HTq_d.ap()[lc - 1])
            htq[lc] = t_
            if lc == 1:
                ht8 = const.tile([128, 4096], F16, tag="ht8")
                nc.sync.dma_start(ht8[:], HT8_d.ap())
                nc.sync.dma_start(hn[2][:], Hn_d.ap()[2])
        nc.sync.dma_start(hn[3][:], Hn_d.ap()[3])

        # ~4us of dummy f32 matmuls while the first DMAs stream in: trips the
        # PE HAM un-throttle (~3.4us sustained busy) so later matmuls run at
        # 2.4GHz instead of the cold 1.2GHz default.
        warm = sp4.tile([128, 512], F32, tag="spart")
        for _ in range(9):
            nc.tensor.matmul(warm[:, :128], ident[:], ident[:],
                             start=True, stop=True)

        hth = {}

        def h_tr(lc, i0, n, dst):
            # build HT d-tiles [i0, i0+n) on-chip from natural tiles
            for ii in range(i0, i0 + n, 2):
                pst = pj.tile([128, 1024], F16, tag="pj")
                for m in range(2):
                    i = ii + m
                    for s in range(4):
                        nc.tensor.transpose(
                            pst[:, (m * 4 + s) * 128:(m * 4 + s + 1) * 128],
                            hn[lc][:, s * 1024 + i * 128:s * 1024 + (i + 1) * 128],
                            identh[:])
                nc.vector.tensor_copy(
                    dst[:, (ii - i0) * 512:(ii - i0 + 2) * 512], pst[:])

        def s_rhs(lc, i):
            if lc == 0:
                return hth[lc][:, i * 512:(i + 1) * 512]
            if lc == 3:
                return ht8[:, i * 512:(i + 1) * 512]
            if i < 4:
                return htq[lc][:, i * 512:(i + 1) * 512]
            return hth[lc][:, (i - 4) * 512:(i - 3) * 512]

        pout = sb.tile([128, NLC * 512], F16, tag="pout")
        ssum = sb.tile([T, NCH], F32, tag="ssum")
        es = {}

        def s_phase(lc):
            if lc == 0:
                hth[lc] = hthf.tile([128, 4096], F16, tag="hthf", name="hthf")
                h_tr(lc, 0, 8, hth[lc])
            elif lc < 3:
                hth[lc] = hthp.tile([128, 2048], F16, tag="hth", name="hth")
                h_tr(lc, 4, 4, hth[lc])
            sps = lg.tile([T, 512], F32, tag="lg")
            if lc == 3:
                # untiled S for the tail chunk: exp reads PSUM directly, no
                # cross-group cast+reduce hops on the critical tail chain
                for i in range(8):
                    nc.tensor.matmul(sps[:], qkt[:, i * 32:(i + 1) * 32],
                                     s_rhs(lc, i), start=(i == 0),
                                     stop=(i == 7 and not with_mask))
            else:
                # --- S[t, l]: 4 col-groups x 2 d-tiles each ---
                spart = sp4.tile([128, 512], F32, tag="spart")
                for k in range(2):
                    for g in range(4):
                        i = g + 4 * k
                        nc.tensor.matmul(spart[32 * g:32 * g + 32, :],
                                         qkt[:, i * 32:(i + 1) * 32],
                                         s_rhs(lc, i),
                                         start=(k == 0), stop=(k == 1),
                                         tile_position=(0, 32 * g))
                sparts = spl.tile([128, 512], F32R, tag="sps")
                nc.vector.tensor_copy(sparts[:], spart[:])
                nc.tensor.matmul(sps[:], sel[:], sparts[:],
                                 start=True, stop=(not with_mask))
            if with_mask:
                nc.tensor.matmul(sps[:], onesr[:],
                                 mbias[:, lc * 512:(lc + 1) * 512],
                                 start=False, stop=True)
            # --- E = exp(S) in fp16, with per-256-chunk row sums ---
            e_ = ep.tile([T, 512], F16, tag="e")
            nc.scalar.activation(e_[:], sps[:], ActFn.Exp)
            for u in range(2):
                c = 2 * lc + u
                nc.vector.reduce_sum(ssum[:, c:c + 1],
                                     e_[:, u * 256:(u + 1) * 256],
                                     axis=mybir.AxisListType.X)
            es[lc] = e_

        ets = {}

        def t_phase(lc):
            # --- E^T fp16: 4 PE transposes packed in one PSUM tile, 1 copy ---
            e_ = es[lc]
            pst = pj.tile([128, 1024], F16, tag="pj")
            for k in range(4):
                nc.tensor.transpose(pst[:, k * 32:(k + 1) * 32],
                                    e_[:, k * 128:(k + 1) * 128],
                                    identh[:32, :32])
            et = etp.tile([128, 128], F16, tag="et")
            nc.vector.tensor_copy(et[:], pst[:, :128])
            ets[lc] = et

        def p_phase(lc):
            # --- P: 4 col-groups j=(cc,h), 2 l-subtiles each ---
            et = ets[lc]
            ppack = zp.tile([128, 512], F32, tag="zp")
            for k in range(2):
                for j in range(4):
                    cc, h = j // 2, j % 2
                    s = 2 * cc + k
                    nc.tensor.matmul(ppack[32 * j:32 * j + 32, :],
                                     et[:, s * 32:(s + 1) * 32],
                                     hn[lc][:, s * 1024 + h * 512:
                                            s * 1024 + h * 512 + 512],
                                     start=(k == 0), stop=(k == 1),
                                     tile_position=(0, 32 * j))
            if lc < 3:
                nc.scalar.activation(pout[:, lc * 512:(lc + 1) * 512], ppack[:],
                                     ActFn.Identity)
            else:
                nc.vector.tensor_copy(pout[:, lc * 512:(lc + 1) * 512], ppack[:])
            # ship this l-chunk's quarter of P as soon as its copy lands
            eng = nc.sync if lc == 3 else nc.scalar
            eng.dma_start(P_d.ap()[:, lc * 512:(lc + 1) * 512],
                          pout[:, lc * 512:(lc + 1) * 512])

        s_phase(0)
        s_phase(1)
        t_phase(0)
        p_phase(0)
        s_phase(2)
        t_phase(1)
        p_phase(1)
        s_phase(3)
        t_phase(2)
        p_phase(2)
        t_phase(3)
        nc.scalar.dma_start(S_d.ap(), ssum[:])
        p_phase(3)

    nc.compile()
    return nc


def kernel(H, G, Wq_core, Wk_core, Wq_win, Wk_win, attn_mask):
    H = np.asarray(H, dtype=np.float32)
    G = np.asarray(G, dtype=np.float32)
    Wq_core = np.asarray(Wq_core, dtype=np.float32)
    Wk_core = np.asarray(Wk_core, dtype=np.float32)
    Wq_win = np.asarray(Wq_win, dtype=np.float32)
    Wk_win = np.asarray(Wk_win, dtype=np.float32)
    mask = np.asarray(attn_mask).astype(bool)

    with_mask = not bool(mask.all())
    key = ("k", with_mask)
    if key not in _CACHE:
        _CACHE[key] = _build(with_mask)
    nc = _CACHE[key]

    # host-side tiny G projections (weight-space only, no H involvement)
    qk = (G @ Wq_core) @ Wk_core.T * S_CORE          # [B, T, DH]
    qw2 = (G @ Wq_win) @ Wk_win.T * S_WIN            # [B, T, DH]
    selmat = np.tile(np.eye(32, dtype=np.float32), (4, 1))  # [128, 32]

    in_maps = []
    for c in range(8):
        b, half = c // 2, c % 2
        l0 = half * SPAN
        H16 = H[b, l0:l0 + SPAN, :].astype(np.float16)          # [2048, 1024]
        HT16 = np.ascontiguousarray(H[b].T[:, l0:l0 + SPAN]).astype(np.float16)
        # [i, p, lc, f] -> [lc, p, i, f]
        HTr = np.ascontiguousarray(
            HT16.reshape(8, 128, NLC, 512).transpose(2, 1, 0, 3)
        ).reshape(NLC, 128, 4096)
        # [j, s, p, f] -> [j, p, s, f]
        Hnr = np.ascontiguousarray(
            H16.reshape(NLC, 4, 128, DH).transpose(0, 2, 1, 3)
        ).reshape(NLC, 128, 4096)
        qkT16 = qk[b].T.astype(np.float16)                      # [1024, 32]
        QKTr = np.ascontiguousarray(
            qkT16.reshape(8, 128, 32).transpose(1, 0, 2)
        ).reshape(128, 256)
        im = {"Hn": Hnr, "HTq": np.ascontiguousarray(HTr[1:3, :, :2048]),
              "HT8": HTr[3], "QKT": QKTr, "SEL": selmat}
        if with_mask:
            im["maskbias"] = np.where(mask[b, l0:l0 + SPAN], 0.0,
                                      -1e9).astype(np.float32)[None, :]
            im["onesrow"] = np.ones((1, T), dtype=np.float32)
        in_maps.append(im)

    import os
    prof_dir = os.environ.get("BGW_PROFILE_DIR")
    res = None
    if prof_dir:
        try:
            res = run_bass_kernel_spmd(nc, in_maps, core_ids=list(range(8)),
                                       trace=True, tmpdir=prof_dir)
        except Exception:
            res = None
    if res is None:
        res = run_bass_kernel_spmd(nc, in_maps, core_ids=list(range(8)))
    kernel._last_result = res

    # ---- host combine: windows = sums of 4 chunk partials, tiny softmax ----
    NCHB = L // 256                                   # 16 chunks per batch
    Z = np.empty((B, T, DH), dtype=np.float32)
    for b in range(B):
        P = np.empty((NCHB, T, DH), dtype=np.float32)
        ss = np.empty((NCHB, T), dtype=np.float32)
        for half in range(2):
            r = res.results[2 * b + half]
            arr = r["P_out"].astype(np.float32).reshape(4, 32, NLC, 512)
            for lc in range(NLC):
                for cc in range(2):
                    for h in range(2):
                        P[half * NCH + 2 * lc + cc, :, h * 512:(h + 1) * 512] = \
                            arr[2 * cc + h, :, lc, :]
            ss[half * NCH:(half + 1) * NCH] = r["ssum_out"].T
        Zw = np.empty((W, T, DH), dtype=np.float32)
        wlog = np.empty((T, W), dtype=np.float32)
        for w in range(W):
            num = P[w] + P[w + 1] + P[w + 2] + P[w + 3]
            den = ss[w] + ss[w + 1] + ss[w + 2] + ss[w + 3]
            Zw[w] = num / den[:, None]
            wlog[:, w] = (Zw[w] * qw2[b]).sum(-1)
        m2 = wlog.max(-1, keepdims=True)
        wsm = np.exp(wlog - m2)
        wsm /= wsm.sum(-1, keepdims=True)
        Z[b] = np.einsum("tw,wtd->td", wsm, Zw)
    return Z
